# revision 29
# baseline (speedup 1.0000x reference)
"""Bass/Trainium2 kernel for nn_EquivariantReynoldsWrap.

The reference module is linear in x: for every pixel,
    out = (1/G) * sum_g BlockDiag(A_g) @ Wf @ BlockDiag(Ainv_g) @ x_pixel
so the whole pipeline collapses into one 64x64 channel-mixing matrix M,
computed on host (cheap). The device work is a single 1x1-conv matmul
out[b] = M @ x[b] with x[b] viewed as (64, H*W).

Sharding: data-parallel over B across the 8 cores (1 batch each).

Per core the 4096-pixel axis is processed in 2 groups of 4 sub-blocks
(512 pixels each). The 4 sub-blocks of a group run as 4 CONCURRENT
64x64 matmuls in the 4 quadrants of the PE array via tile_position
(measured dstart ~7ns between quadrants), with M^T loaded in both
partition halves of the stationary operand.

Raw bacc (no TileContext): hand-rolled semaphores, minimal head/tail.
Key HW subtleties handled below:
 - one semaphore per DMA (per-engine incs of two DMAs on a ring mix);
 - DMA triggers are sequencer-class: they need the producer's
   completion sem, queue order alone is not enough;
 - a matmul's sem update fires at retire, before the ~128-cycle
   systolic drain lands in PSUM: a short guard matmul carries the inc;
 - DRAM-side AP outer dim must be large (64) to spread a transfer
   across all 16 SDMA engines.
"""

import numpy as np

import concourse.bacc as bacc
import concourse.bass as bass
from concourse import mybir
from concourse.bass_utils import run_bass_kernel_spmd

B, C, H, W_SP = 8, 64, 64, 64
COUT = 64
HW = H * W_SP          # 4096 pixels per batch
N_CORES = 8

N_WARM = 5             # bf16 warm-up matmuls (HAM un-throttle)
USE_F32R = False       # single-pass matmuls, ~1.5e-4 rel err (vs 1e-7 fp32)

TRACE = False          # test.py flips this to profile
_cached_nc = None


def _build_nc():
    global _cached_nc
    if _cached_nc is not None:
        return _cached_nc

    in_dt = mybir.dt.float32r if USE_F32R else mybir.dt.float32
    f32 = mybir.dt.float32

    nc = bacc.Bacc(
        "TRN2",
        target_bir_lowering=False,
        debug=False,
        enable_asserts=False,
        num_devices=N_CORES,
    )
    xdh = nc.dram_tensor("x", [C, HW], in_dt, kind="ExternalInput")
    wdh = nc.dram_tensor("w", [128, 64], in_dt, kind="ExternalInput")
    ydh = nc.dram_tensor("y", [COUT, HW], f32, kind="ExternalOutput")
    xd = xdh.ap()
    wd = wdh.ap()

    with (
        nc.sbuf_tensor("wt", [128, 64], in_dt) as wt_t,
        nc.sbuf_tensor("xt", [128, 2048], in_dt) as xt_t,
        nc.sbuf_tensor("ot", [128, 2048], f32) as ot_t,
        nc.sbuf_tensor("zt", [128, 384], mybir.dt.bfloat16) as zt_t,
        nc.psum_tensor([128, 2048], f32) as ps_t,
        nc.psum_tensor([128, 384], f32) as wps_t,
        nc.semaphore("s_w") as s_w,
        nc.semaphore("s_i0l") as s_i0l,  # x group 0, partitions 0-63
        nc.semaphore("s_i0h") as s_i0h,  # x group 0, partitions 64-127
        nc.semaphore("s_i1l") as s_i1l,
        nc.semaphore("s_i1h") as s_i1h,
        nc.semaphore("s_z") as s_z,
        nc.semaphore("s_mm") as s_mm,
        nc.semaphore("s_cpv") as s_cpv,
        nc.semaphore("s_cpa") as s_cpa,
        nc.semaphore("s_y") as s_y,
        nc.Block() as block,
    ):
        wt = wt_t.ap()
        xt = xt_t.ap()
        ot = ot_t.ap()
        zt = zt_t.ap()
        ps = ps_t.ap()
        wps = wps_t.ap()

        # Group g covers pixels [g*2048, (g+1)*2048) as 4 sub-blocks of
        # 512: sb0..sb3. The low-partition DMA loads pixels sb0|sb1 into
        # parts 0-63 (cols lo|hi of the group's xt range); the high one
        # loads sb2|sb3 into parts 64-127.
        # Quadrant (0,0):  sb0 -> psP parts 0-63
        # Quadrant (64,64):sb2 -> psP parts 64-127
        # Quadrant (0,64): sb1 -> psQ parts 64-127
        # Quadrant (64,0): sb3 -> psQ parts 0-63
        # ot after copies: parts 0-63 = {sb0@lo, sb3@hi},
        #                  parts 64-127 = {sb2@lo, sb1@hi}

        def psP(g):
            return ps[:, g * 512:(g + 1) * 512]           # banks 0,1

        def psQ(g):
            return ps[:, 1024 + g * 512:1024 + (g + 1) * 512]  # banks 2,3

        def xcols(g, hi):
            a = g * 1024 + (512 if hi else 0)
            return slice(a, a + 512)

        def ocols(g, hi):
            a = g * 1024 + (512 if hi else 0)
            return slice(a, a + 512)

        # out DMAs per (group, partition half), strided DRAM APs:
        # low half  -> blocks {sb0, sb3}: offsets g*2048 + {0, 1536}
        # high half -> blocks {sb2, sb1}: offsets g*2048 + {1024, 512}
        def y_low_ap(g):
            return bass.AP(ydh, g * 2048, [[HW, 64], [1536, 2], [1, 512]])

        def y_high_ap(g):
            return bass.AP(
                ydh, g * 2048 + 1024, [[HW, 64], [-512, 2], [1, 512]]
            )

        def ot_low(g):
            # sb0 at ot[0:64, g*1024:+512], sb3 at ot[0:64, g*1024+512:+512]
            return ot[0:64, g * 1024:(g + 1) * 1024]

        def ot_high(g):
            return ot[64:128, g * 1024:(g + 1) * 1024]

        @block.sync
        def _(sync):
            # weights (small) first, then x low-partition halves; each
            # ring's FIRST x chunk feeds group 0 (ring DMAs complete
            # serially, so later DMAs gate later groups)
            sync.dma_start(wt[:], wd[:]).then_inc(s_w, 16)
            sync.dma_start(
                xt[0:64, 0:1024], xd[:, 0:1024]
            ).then_inc(s_i0l, 16)
            sync.dma_start(
                xt[0:64, 1024:2048], xd[:, 2048:3072]
            ).then_inc(s_i1l, 16)
            sync.wait_ge(s_cpv, 1)
            sync.wait_ge(s_cpa, 1)
            sync.dma_start(y_low_ap(0), ot_low(0)).then_inc(s_y, 16)
            sync.wait_ge(s_cpv, 2)
            sync.wait_ge(s_cpa, 2)
            sync.dma_start(y_low_ap(1), ot_low(1)).then_inc(s_y, 16)
            # hold the program open until every output write landed; the
            # walrus epilogue then zeroes all semaphores for re-execution
            sync.wait_ge(s_y, 64)

        @block.scalar
        def _(scalar):
            # (bacc hoists the ACT table load to the top of this block)
            scalar.dma_start(
                xt[64:128, 0:1024], xd[:, 1024:2048]
            ).then_inc(s_i0h, 16)
            scalar.dma_start(
                xt[64:128, 1024:2048], xd[:, 3072:4096]
            ).then_inc(s_i1h, 16)
            scalar.wait_ge(s_mm, 1)
            scalar.copy(ot[:, ocols(0, True)], psQ(0)).then_inc(s_cpa)
            scalar.wait_ge(s_cpv, 1)
            scalar.wait_ge(s_cpa, 1)
            scalar.dma_start(y_high_ap(0), ot_high(0)).then_inc(s_y, 16)
            scalar.wait_ge(s_mm, 2)
            scalar.copy(ot[:, ocols(1, True)], psQ(1)).then_inc(s_cpa)
            scalar.wait_ge(s_cpv, 2)
            scalar.wait_ge(s_cpa, 2)
            scalar.dma_start(y_high_ap(1), ot_high(1)).then_inc(s_y, 16)

        @block.tensor
        def _(tensor):
            # HAM warm-up on zeroed bf16 tile (1 HW pass each)
            tensor.wait_ge(s_z, 1)
            for _ in range(N_WARM):
                tensor.matmul(wps[:], zt[:, :128], zt[:])

            def group(g):
                lo, hi = xcols(g, False), xcols(g, True)
                tensor.matmul(psP(g)[0:64, :], wt[0:64, :],
                              xt[0:64, lo], tile_position=(0, 0))
                tensor.matmul(psP(g)[64:128, :], wt[64:128, :],
                              xt[64:128, lo], tile_position=(64, 64))
                tensor.matmul(psQ(g)[64:128, :], wt[0:64, :],
                              xt[0:64, hi], tile_position=(0, 64))
                tensor.matmul(psQ(g)[0:64, :], wt[64:128, :],
                              xt[64:128, hi], tile_position=(64, 0))
                # guard: the quadrant matmuls' sem updates fire at retire,
                # before their systolic drains land in PSUM; this
                # full-array matmul retires after all drains are covered
                tensor.matmul(wps[:], zt[:, :128], zt[:]).then_inc(s_mm)

            tensor.wait_ge(s_w, 16)
            tensor.wait_ge(s_i0l, 16)
            tensor.wait_ge(s_i0h, 16)
            group(0)
            tensor.wait_ge(s_i1l, 16)
            tensor.wait_ge(s_i1h, 16)
            group(1)

        @block.vector
        def _(vector):
            vector.wait_ge(s_mm, 1)
            vector.tensor_copy(ot[:, ocols(0, False)], psP(0)).then_inc(s_cpv)
            vector.wait_ge(s_mm, 2)
            vector.tensor_copy(ot[:, ocols(1, False)], psP(1)).then_inc(s_cpv)

        @block.gpsimd
        def _(gpsimd):
            gpsimd.memset(zt[:], 0.0).then_inc(s_z)

    nc.compile()
    _cached_nc = nc
    return nc


def _fuse_weights(group_tensor, group_tensor_inv, Wf):
    A = np.asarray(group_tensor, np.float64)
    Ai = np.asarray(group_tensor_inv, np.float64)
    Wf64 = np.asarray(Wf, np.float64)
    G, CG, _ = A.shape
    n = C // CG
    eye = np.eye(n)
    M = np.zeros((COUT, C))
    for g in range(G):
        M += np.kron(eye, A[g]) @ Wf64 @ np.kron(eye, Ai[g])
    M /= G
    MT = np.ascontiguousarray(M.T).astype(np.float32)
    # M^T replicated in both partition halves for the two row-quadrants
    return np.concatenate([MT, MT], axis=0)  # [128, 64]


def kernel(x, group_tensor, group_tensor_inv, Wf):
    nc = _build_nc()
    WQ = _fuse_weights(group_tensor, group_tensor_inv, Wf)
    x = np.ascontiguousarray(np.asarray(x, np.float32))

    in_maps = [
        {"x": x[b].reshape(C, HW), "w": WQ} for b in range(B)
    ]
    res = run_bass_kernel_spmd(
        nc, in_maps, core_ids=list(range(N_CORES)), trace=TRACE
    )
    if TRACE:
        kernel.last_results = res
    y = np.stack(
        [res.results[b]["y"].reshape(COUT, H, W_SP) for b in range(B)]
    )
    return y


# revision 30
# speedup vs baseline: 1.1646x; 1.1646x over previous
"""Bass/Trainium2 kernel for nn_EquivariantReynoldsWrap.

The reference module is linear in x: for every pixel,
    out = (1/G) * sum_g BlockDiag(A_g) @ Wf @ BlockDiag(Ainv_g) @ x_pixel
so the whole pipeline collapses into one 64x64 channel-mixing matrix M,
computed on host (cheap). The device work is a single 1x1-conv matmul
out[b] = M @ x[b] with x[b] viewed as (64, H*W).

Sharding: data-parallel over B across the 8 cores (1 batch each).
Per core the two halves of the pixel axis are interleaved on the
partition axis (partition p = channel p//2, half p%2) and the stationary
weight is the 128x128 interleaved block-diagonal of M^T, so each
512-column matmul covers 1024 pixels.

Raw bacc (no TileContext): hand-rolled semaphores, minimal head/tail.
"""

import numpy as np

import concourse.bacc as bacc
import concourse.bass as bass
from concourse import mybir
from concourse.bass_utils import run_bass_kernel_spmd

B, C, H, W_SP = 8, 64, 64, 64
COUT = 64
HW = H * W_SP          # 4096 pixels per batch
HALF = HW // 2         # 2048 -> stacked column count per core
N_CORES = 8

CH = 512               # columns per pipeline chunk
N_CHUNKS = HALF // CH  # 4
N_WARM = 8             # bf16 warm-up matmuls (HAM un-throttle)
USE_F32R = False       # single-pass matmuls, ~1.5e-4 rel err (vs 1e-7 fp32)

TRACE = False          # test.py flips this to profile
_cached_nc = None


def _build_nc():
    global _cached_nc
    if _cached_nc is not None:
        return _cached_nc

    in_dt = mybir.dt.float32r if USE_F32R else mybir.dt.float32
    f32 = mybir.dt.float32

    nc = bacc.Bacc(
        "TRN2",
        target_bir_lowering=False,
        debug=False,
        enable_asserts=False,
        num_devices=N_CORES,
    )
    xd = nc.dram_tensor("x", [C, HW], in_dt, kind="ExternalInput").ap()
    wd = nc.dram_tensor("w", [128, 128], in_dt, kind="ExternalInput").ap()
    yd = nc.dram_tensor("y", [COUT, HW], f32, kind="ExternalOutput").ap()

    # [64, 2, t] c-major outer dims: the DMA pairs partition p with
    # (c=p//2, s=p%2); the outer dim of 64 spreads each transfer across
    # all 16 SDMA engines (an outer dim of 2 used only 2 of them).
    xr = xd.rearrange("c (s t) -> c s t", s=2)
    yr = yd.rearrange("c (s t) -> c s t", s=2)

    with (
        nc.sbuf_tensor("wt", [128, 128], in_dt) as wt_t,
        nc.sbuf_tensor("xt", [128, HALF], in_dt) as xt_t,
        nc.sbuf_tensor("ot", [128, HALF], f32) as ot_t,
        nc.sbuf_tensor("zt", [128, 512], mybir.dt.bfloat16) as zt_t,
        nc.psum_tensor([128, HALF], f32) as ps_t,
        nc.psum_tensor([128, 512], f32) as wps_t,
        nc.semaphore("s_w") as s_w,      # weights DMA done
        # one sem per x-chunk DMA: a sem shared by two DMAs on one ring
        # reaches 16 from a MIX of the two transfers' per-engine incs
        nc.semaphore("s_x0") as s_x0,
        nc.semaphore("s_x1") as s_x1,
        nc.semaphore("s_x2") as s_x2,
        nc.semaphore("s_x3") as s_x3,
        nc.semaphore("s_z") as s_z,      # warmup tile zeroed
        nc.semaphore("s_mm") as s_mm,    # matmul per chunk
        nc.semaphore("s_cpv") as s_cpv,  # DVE copies (chunks 0, 2)
        nc.semaphore("s_cpa") as s_cpa,  # ACT copies (chunks 1, 3)
        nc.semaphore("s_y") as s_y,      # out DMAs
        nc.Block() as block,
    ):
        wt = wt_t.ap()
        xt = xt_t.ap()
        ot = ot_t.ap()
        zt = zt_t.ap()
        ps = ps_t.ap()
        wps = wps_t.ap()

        def cs(i):
            return slice(i * CH, (i + 1) * CH)

        @block.sync
        def _(sync):
            # x chunks 0, 2 on the SP HWDGE ring
            sync.dma_start(xt[:, cs(0)], xr[:, :, cs(0)]).then_inc(s_x0, 16)
            sync.dma_start(xt[:, cs(2)], xr[:, :, cs(2)]).then_inc(s_x2, 16)
            sync.wait_ge(s_cpv, 1)
            sync.dma_start(yr[:, :, cs(0)], ot[:, cs(0)]).then_inc(s_y, 16)
            sync.wait_ge(s_cpv, 2)
            sync.dma_start(yr[:, :, cs(2)], ot[:, cs(2)]).then_inc(s_y, 16)
            # hold the program open until every output write landed; the
            # walrus epilogue then zeroes all semaphores for re-execution
            sync.wait_ge(s_y, 64)

        @block.scalar
        def _(scalar):
            # weights + x chunks 1, 3 on the ACT HWDGE ring
            # (bacc hoists the ACT table load to the top of this block)
            scalar.dma_start(wt[:], wd[:]).then_inc(s_w, 16)
            scalar.dma_start(xt[:, cs(1)], xr[:, :, cs(1)]).then_inc(s_x1, 16)
            scalar.dma_start(xt[:, cs(3)], xr[:, :, cs(3)]).then_inc(s_x3, 16)
            # copies for chunks 1, 3; the out DMA trigger is a
            # sequencer-class op, so it must gate on the copy's completion
            # sem (queue order alone does NOT order it after the datapath)
            scalar.wait_ge(s_mm, 2)
            scalar.copy(ot[:, cs(1)], ps[:, cs(1)]).then_inc(s_cpa)
            scalar.wait_ge(s_cpa, 1)
            scalar.dma_start(yr[:, :, cs(1)], ot[:, cs(1)]).then_inc(s_y, 16)
            scalar.wait_ge(s_mm, 4)
            scalar.copy(ot[:, cs(3)], ps[:, cs(3)]).then_inc(s_cpa)
            scalar.wait_ge(s_cpa, 2)
            scalar.dma_start(yr[:, :, cs(3)], ot[:, cs(3)]).then_inc(s_y, 16)

        @block.tensor
        def _(tensor):
            # HAM warm-up on zeroed bf16 tile (1 HW pass each)
            tensor.wait_ge(s_z, 1)
            for _ in range(N_WARM):
                tensor.matmul(wps[:], zt[:, :128], zt[:])

            def guarded_mm(i):
                # The matmul's sem update fires at instruction retire (last
                # column ENTERS the array); the ~128-cycle systolic drain is
                # still writing PSUM then. Put the inc on a short guard
                # matmul so waiters can't catch the drain window.
                tensor.matmul(ps[:, cs(i)], wt[:], xt[:, cs(i)])
                tensor.matmul(wps[:, :256], zt[:, :128], zt[:, :256]).then_inc(s_mm)

            tensor.wait_ge(s_w, 16)
            tensor.wait_ge(s_x0, 16)
            guarded_mm(0)
            tensor.wait_ge(s_x1, 16)
            guarded_mm(1)
            tensor.wait_ge(s_x2, 16)
            guarded_mm(2)
            tensor.wait_ge(s_x3, 16)
            guarded_mm(3)

        @block.vector
        def _(vector):
            vector.wait_ge(s_mm, 1)
            vector.tensor_copy(ot[:, cs(0)], ps[:, cs(0)]).then_inc(s_cpv)
            vector.wait_ge(s_mm, 3)
            vector.tensor_copy(ot[:, cs(2)], ps[:, cs(2)]).then_inc(s_cpv)

        @block.gpsimd
        def _(gpsimd):
            gpsimd.memset(zt[:], 0.0).then_inc(s_z)

    nc.compile()
    _cached_nc = nc
    return nc


def _fuse_weights(group_tensor, group_tensor_inv, Wf):
    A = np.asarray(group_tensor, np.float64)
    Ai = np.asarray(group_tensor_inv, np.float64)
    Wf64 = np.asarray(Wf, np.float64)
    G, CG, _ = A.shape
    n = C // CG
    eye = np.eye(n)
    M = np.zeros((COUT, C))
    for g in range(G):
        M += np.kron(eye, A[g]) @ Wf64 @ np.kron(eye, Ai[g])
    M /= G
    MT = np.ascontiguousarray(M.T).astype(np.float32)
    # interleaved packing: x-tile partition p holds channel p//2 of pixel
    # half p%2; out partition q holds channel q//2 of half q%2.
    W2T = np.zeros((128, 128), np.float32)
    W2T[0::2, 0::2] = MT
    W2T[1::2, 1::2] = MT
    return W2T


def kernel(x, group_tensor, group_tensor_inv, Wf):
    nc = _build_nc()
    W2T = _fuse_weights(group_tensor, group_tensor_inv, Wf)
    x = np.ascontiguousarray(np.asarray(x, np.float32))

    in_maps = [
        {"x": x[b].reshape(C, HW), "w": W2T} for b in range(B)
    ]
    res = run_bass_kernel_spmd(
        nc, in_maps, core_ids=list(range(N_CORES)), trace=TRACE
    )
    if TRACE:
        kernel.last_results = res
    y = np.stack(
        [res.results[b]["y"].reshape(COUT, H, W_SP) for b in range(B)]
    )
    return y
